# revision 10
# baseline (speedup 1.0000x reference)
"""Trainium2 Bass kernel for the SCAN cross-attention contrastive loss.

Math (validated against the reference):
  For caption c with words zeroed beyond its true length, images pre-scaled
  on host by 1/||img_ir|| (column scale commutes through the whole chain):
    A[w, ir]   = <recipes[c, w, :], images_flat[ir, :]/||img_ir||>
    L          = leaky_relu(A, 0.1)       (zero rows stay zero)
    rinv[ir]   = 9 / ||L[:, ir]||         (= exp(-0.5*ln(nrm2/81)))
    E          = exp(L * rinv)            (softmax numerator; denominator
                                           cancels in the cosine below)
    num[ir]    = sum_w E * A              (already has the 1/||img|| folded)
    u2[ir]     = sum_w E * (G @ E),  G = R_c R_c^T
    rs[ir]     = num / sqrt(u2)           (= row_sim of the reference)
    score[c,i] = sum_{r in image i} exp(6 * rs)   (log/6 applied on host)
  Final hinge-loss reduction over the 96x96 score matrix is done on host.

Sharding: captions sorted by length, dealt round-robin to 8 cores (slot s of
core k = sorted[s*8+k]) so every core shares the same per-slot padded length,
then slots are FFD-bin-packed into <=128-partition groups. One SPMD program;
per-core data (packed recipes, Gram blocks, masks) differs.

Structure (per core): fused loop over (chunk j of 432 ir-columns) x (group g).
Pass 1 per j: A matmul -> Prelu(L) -> Lsq -> nrm2 accumulated via capmask
matmul into partition rows 12j..12j+12 of one stacked [96,432] PSUM tile.
rinv per j on the [12,432] slice; broadcast to word rows via stride-0 DMA.
Pass 2 per j: T = L*rb, E = exp(T), F = G@E, P1 = E*A, P2 = E*F, then capmask
matmuls accumulate num/u2 into stacked [96,432] PSUM tiles (rows 12j..).
One epilogue on the stacked [96,432] tiles replaces 8 per-chunk epilogues.
"""

import sys

sys.path.insert(0, "/opt/trn_rl_repo")

import numpy as np

I, R, D = 96, 36, 256
C, W = 96, 48
IR = I * R  # 3456
N_CORES = 8
CPC = C // N_CORES  # captions per core = 12
CHUNK = 432  # IR columns per iteration = 12 images = one PSUM bank
N_CHUNKS = IR // CHUNK  # 8
IMG_PER_CHUNK = CHUNK // R  # 12

_CACHE = {}


def _plan_groups(cap_lens):
    """Slot lengths (max over cores per round-robin slot) + FFD packing."""
    order = np.argsort(-cap_lens, kind="stable")  # longest first
    slot_len = [int(cap_lens[order[s * N_CORES]]) for s in range(CPC)]
    bins = []  # [rows_used, [slot indices]]
    for s in range(CPC):
        ln = slot_len[s]
        for b in bins:
            if b[0] + ln <= 128:
                b[1].append(s)
                b[0] += ln
                break
        else:
            bins.append([ln, [s]])
    groups = []
    for rows, slots in bins:
        offs = []
        off = 0
        for s in slots:
            offs.append(off)
            off += slot_len[s]
        groups.append({"slots": slots, "offs": offs, "P": rows})
    return order, slot_len, groups


def _patch_act_tables():
    """Pin every activation we use to the natural_log_exp_and_others table
    set so the kernel needs exactly one ACT_TABLE_LOAD."""
    import concourse.hw_specs as hw_specs

    if getattr(hw_specs, "_act_tables_pinned", False):
        return
    orig = hw_specs.get_activation_tables

    def pinned(module_arch):
        tables = orig(module_arch)
        keep = "natural_log_exp_and_others"
        if keep in tables:
            shared = tables[keep]
            for name, funcs in tables.items():
                if name != keep:
                    tables[name] = funcs - shared
        return tables

    hw_specs.get_activation_tables = pinned
    import concourse.bacc as bacc_mod
    if getattr(bacc_mod, "get_activation_tables", None) is orig:
        bacc_mod.get_activation_tables = pinned
    hw_specs._act_tables_pinned = True


def _build_program(cap_lens):
    import concourse.bacc as bacc
    import concourse.mybir as mybir
    from concourse.tile import TileContext

    _patch_act_tables()

    fp32 = mybir.dt.float32
    f16 = mybir.dt.float16
    bf16 = mybir.dt.bfloat16
    ACT = mybir.ActivationFunctionType
    ALU = mybir.AluOpType
    AX = mybir.AxisListType

    order, slot_len, groups = _plan_groups(cap_lens)
    NG = len(groups)

    nc = bacc.Bacc("TRN2", target_bir_lowering=False, debug=False,
                   num_devices=N_CORES)

    imagesT_d = nc.dram_tensor("imagesT", [2, 128, IR], f16,
                               kind="ExternalInput")
    recT_d, G_d, cmh_d, cmb_d = [], [], [], []
    for g, gr in enumerate(groups):
        P = gr["P"]
        recT_d.append(nc.dram_tensor(f"recT{g}", [2, 128, P], f16,
                                     kind="ExternalInput"))
        G_d.append(nc.dram_tensor(f"G{g}", [P, P], f16, kind="ExternalInput"))
        cmh_d.append(nc.dram_tensor(f"cmh{g}", [P, CPC], f16,
                                    kind="ExternalInput"))
        # per-chunk stationary [P, 96] with the capmask block shifted to
        # columns 12j so the num/u2 matmuls write partition base 0 of the
        # stacked accumulators (PE requires out base partition 0/32/64)
        cmb_d.append(nc.dram_tensor(f"cmb{g}", [P, N_CHUNKS * C], bf16,
                                    kind="ExternalInput"))
    out_d = nc.dram_tensor("scores", [C, IMG_PER_CHUNK], fp32,
                           kind="ExternalOutput")

    with TileContext(nc) as tc:
        with (
            tc.tile_pool(name="const", bufs=1) as cpool,
            tc.tile_pool(name="work", bufs=4) as wpool,
            tc.tile_pool(name="rb", bufs=2) as rbpool,
            tc.tile_pool(name="small", bufs=2) as spool,
            tc.tile_pool(name="dscr", bufs=2, space="DRAM") as dpool,
            tc.tile_pool(name="psA", bufs=3, space="PSUM") as psA,
            tc.tile_pool(name="psF", bufs=2, space="PSUM") as psF,
            tc.tile_pool(name="psN", bufs=1, space="PSUM") as psN,
            tc.tile_pool(name="psAcc", bufs=1, space="PSUM") as psAcc,
        ):
            # ---- resident constants ----
            # group-0 constants first so iteration (j=0, g=0) can start early
            recT, Gt, cmh, cmb = [None] * NG, [None] * NG, [None] * NG, [None] * NG
            for g, gr in enumerate(groups):
                P = gr["P"]
                rt = cpool.tile([128, 2 * P], f16, tag=f"recT{g}")
                for kc in range(2):
                    nc.gpsimd.dma_start(out=rt[:, kc * P:(kc + 1) * P],
                                        in_=recT_d[g][kc, :, :])
                recT[g] = rt
                mt = cpool.tile([P, CPC], f16, tag=f"cmh{g}")
                nc.gpsimd.dma_start(out=mt[:, :], in_=cmh_d[g][:, :])
                cmh[g] = mt
                gt = cpool.tile([P, P], f16, tag=f"G{g}")
                nc.gpsimd.dma_start(out=gt[:, :], in_=G_d[g][:, :])
                Gt[g] = gt
                mb = cpool.tile([P, N_CHUNKS * C], bf16, tag=f"cmb{g}")
                nc.gpsimd.dma_start(out=mb[:, :], in_=cmb_d[g][:, :])
                cmb[g] = mb
            # images: per-chunk pieces so compute starts after the first lands
            imgT = cpool.tile([128, 2 * IR], f16, tag="imgT")
            for j in range(N_CHUNKS):
                for kc in range(2):
                    nc.gpsimd.dma_start(
                        out=imgT[:, kc * IR + j * CHUNK:
                                 kc * IR + (j + 1) * CHUNK],
                        in_=imagesT_d[kc, :, j * CHUNK:(j + 1) * CHUNK])

            scores = cpool.tile([C, IMG_PER_CHUNK], fp32, tag="scores")
            accU = psAcc.tile([C, CHUNK], fp32, tag="accU")
            accM = psAcc.tile([C, CHUNK], fp32, tag="accM")

            for j in range(N_CHUNKS):
                j0 = j * CHUNK
                c0 = j * C
                As, Ls = [], []
                accN = psN.tile([CPC, CHUNK], fp32, tag="accN")
                # ---- pass 1: raw attention, leaky, norms ----
                for g, gr in enumerate(groups):
                    P = gr["P"]
                    A_ps = psA.tile([128, CHUNK], fp32, tag="A")
                    for kc in range(2):
                        nc.tensor.matmul(
                            A_ps[:P, :],
                            recT[g][:, kc * P:(kc + 1) * P],
                            imgT[:, kc * IR + j0:kc * IR + j0 + CHUNK],
                            start=(kc == 0), stop=(kc == 1))
                    As.append(A_ps)
                    L = wpool.tile([128, CHUNK], f16, tag=f"L{g}")
                    nc.scalar.activation(L[:P, :], A_ps[:P, :], ACT.Prelu,
                                         alpha=0.1)
                    Ls.append(L)
                    Lsq = wpool.tile([128, CHUNK], f16, tag="Lsq")
                    nc.gpsimd.tensor_mul(Lsq[:P, :], L[:P, :], L[:P, :])
                    nc.tensor.matmul(accN[:, :], cmh[g][:, :],
                                     Lsq[:P, :], start=(g == 0),
                                     stop=(g == NG - 1))
                # ---- rinv for this chunk: 9/sqrt(nrm2) ----
                lnt = spool.tile([CPC, CHUNK], fp32, tag="lnt")
                nc.scalar.activation(lnt[:, :], accN[:, :], ACT.Ln,
                                     scale=1.0 / 81.0)
                rinv = spool.tile([CPC, CHUNK], f16, tag="rinv")
                nc.scalar.activation(rinv[:, :], lnt[:, :], ACT.Exp,
                                     scale=-0.5)
                # broadcast rinv rows to word rows: bounce through a DRAM
                # scratch (SBUF DMA APs forbid stride-0 partition dims; DRAM
                # APs don't), then stride-0 read per slot
                rscr = dpool.tile([CPC, CHUNK], f16, tag="rscr")
                nc.gpsimd.dma_start(out=rscr[:, :], in_=rinv[:, :])
                rbs = []
                for g, gr in enumerate(groups):
                    P = gr["P"]
                    rb = rbpool.tile([128, CHUNK], f16, tag=f"rb{g}")
                    for s, off in zip(gr["slots"], gr["offs"]):
                        lp = slot_len[s]
                        nc.gpsimd.dma_start(
                            out=rb[off:off + lp, :],
                            in_=rscr[s:s + 1, :].to_broadcast([lp, CHUNK]))
                    rbs.append(rb)
                # ---- pass 2: softmax numerator, Gram products, reductions ----
                for g, gr in enumerate(groups):
                    P = gr["P"]
                    T = wpool.tile([128, CHUNK], f16, tag="T")
                    if g == 0:
                        nc.vector.tensor_mul(T[:P, :], Ls[g][:P, :],
                                             rbs[g][:P, :])
                    else:
                        nc.gpsimd.tensor_mul(T[:P, :], Ls[g][:P, :],
                                             rbs[g][:P, :])
                    E = wpool.tile([128, CHUNK], f16, tag="E")
                    nc.scalar.activation(E[:P, :], T[:P, :], ACT.Exp)
                    F_ps = psF.tile([128, CHUNK], fp32, tag="F")
                    nc.tensor.matmul(F_ps[:P, :], Gt[g][:, :], E[:P, :],
                                     start=True, stop=True)
                    P1 = wpool.tile([128, CHUNK], bf16, tag="P1")
                    nc.vector.tensor_mul(P1[:P, :], E[:P, :], As[g][:P, :])
                    P2 = wpool.tile([128, CHUNK], bf16, tag="P2")
                    nc.vector.tensor_mul(P2[:P, :], E[:P, :], F_ps[:P, :])
                    nc.tensor.matmul(accM[:, :], cmb[g][:, c0:c0 + C],
                                     P1[:P, :], start=(j == 0 and g == 0),
                                     stop=(j == N_CHUNKS - 1 and g == NG - 1))
                    nc.tensor.matmul(accU[:, :], cmb[g][:, c0:c0 + C],
                                     P2[:P, :], start=(j == 0 and g == 0),
                                     stop=(j == N_CHUNKS - 1 and g == NG - 1))
            # ---- one stacked epilogue over all (caption, chunk) rows ----
            # rs = num * rsqrt(u2); scores = sum_r exp(6*rs)
            lw = spool.tile([C, CHUNK], fp32, tag="lw")
            nc.scalar.activation(lw[:, :], accU[:, :], ACT.Ln)
            q = spool.tile([C, CHUNK], fp32, tag="q")
            nc.scalar.activation(q[:, :], lw[:, :], ACT.Exp, scale=-0.5)
            rs = spool.tile([C, CHUNK], fp32, tag="rs")
            nc.vector.tensor_mul(rs[:, :], accM[:, :], q[:, :])
            e6 = spool.tile([C, CHUNK], fp32, tag="e6")
            nc.scalar.activation(e6[:, :], rs[:, :], ACT.Exp, scale=6.0)
            nc.vector.tensor_reduce(
                scores[:, :],
                e6[:, :].rearrange("p (i r) -> p i r", r=R),
                axis=AX.X, op=ALU.add)
            nc.gpsimd.dma_start(out=out_d[:, :], in_=scores[:, :])
    nc.compile()
    return nc, order, slot_len, groups


def _host_inputs(images, recipes, cap_lens, order, slot_len, groups):
    import ml_dtypes

    f16 = np.float16
    imgf = images.reshape(IR, D).astype(np.float64)
    n1 = np.sqrt((imgf ** 2).sum(axis=1))
    imgs = (imgf / np.maximum(n1, 1e-8)[:, None]).astype(np.float32)
    imagesT = np.ascontiguousarray(imgs.T).reshape(2, 128, IR).astype(f16)

    in_maps = []
    for k in range(N_CORES):
        m = {"imagesT": imagesT}
        for g, gr in enumerate(groups):
            P = gr["P"]
            Rg = np.zeros((P, D), np.float32)  # packed, zero-padded recipes
            cm = np.zeros((P, CPC), np.float32)
            Gm = np.zeros((P, P), np.float32)
            for s, off in zip(gr["slots"], gr["offs"]):
                cap = int(order[s * N_CORES + k])
                ln = int(cap_lens[cap])
                lp = slot_len[s]
                rws = recipes[cap, :ln, :].astype(np.float32)
                Rg[off:off + ln, :] = rws
                Gm[off:off + ln, off:off + ln] = rws @ rws.T
                cm[off:off + lp, s] = 1.0
            m[f"recT{g}"] = np.ascontiguousarray(Rg.T).reshape(
                2, 128, P).astype(f16)
            m[f"G{g}"] = Gm.astype(f16)
            m[f"cmh{g}"] = cm.astype(f16)
            # stacked per-chunk stationaries: chunk j's mask block lives at
            # columns j*96 + 12j .. so its matmul writes rows 12j..12j+12 of
            # the stacked [96, CHUNK] accumulators (base partition 0)
            cms = np.zeros((P, N_CHUNKS * C), np.float32)
            for j in range(N_CHUNKS):
                cms[:, j * C + j * CPC: j * C + (j + 1) * CPC] = cm
            m[f"cmb{g}"] = cms.astype(ml_dtypes.bfloat16)
        in_maps.append(m)
    return in_maps


def run_sharded(images, recipes, cap_lens, **spmd_kwargs):
    """Compile (cached), run on 8 cores, return (sumexp (C, I) fp64, results)."""
    from concourse.bass_utils import run_bass_kernel_spmd

    cap_lens = np.asarray(cap_lens).astype(np.int32)
    key = cap_lens.tobytes()
    if key not in _CACHE:
        _CACHE[key] = _build_program(cap_lens)
    nc, order, slot_len, groups = _CACHE[key]

    in_maps = _host_inputs(np.asarray(images), np.asarray(recipes), cap_lens,
                           order, slot_len, groups)
    res = run_bass_kernel_spmd(nc, in_maps, list(range(N_CORES)), **spmd_kwargs)

    sumexp = np.zeros((C, I), np.float64)
    for k in range(N_CORES):
        sc = res.results[k]["scores"].astype(np.float64)  # [96, 12]
        for s in range(CPC):
            cap = int(order[s * N_CORES + k])
            for j in range(N_CHUNKS):
                sumexp[cap, j * IMG_PER_CHUNK:(j + 1) * IMG_PER_CHUNK] = \
                    sc[j * CPC + s]
    return sumexp, res


def kernel(images, recipes, cap_lens):
    sumexp, _ = run_sharded(images, recipes, cap_lens)
    S = (np.log(sumexp) / 6.0).T  # (I, C)
    diag = np.diag(S)
    eye = np.eye(I, dtype=bool)
    ci = np.where(eye, 0.0, np.maximum(0.2 + S - diag[None, :], 0.0))
    cr = np.where(eye, 0.0, np.maximum(0.2 + S - diag[:, None], 0.0))
    return np.float32(ci.sum() + cr.sum())


# revision 18
# speedup vs baseline: 1.1892x; 1.1892x over previous
"""Trainium2 Bass kernel for the SCAN cross-attention contrastive loss.

Math (validated against the reference):
  For caption c with words zeroed beyond its true length, images pre-scaled
  on host by 1/||img_ir|| (column scale commutes through the whole chain):
    A[w, ir]   = <recipes[c, w, :], images_flat[ir, :]/||img_ir||>
    L          = leaky_relu(A, 0.1)       (zero rows stay zero)
    rinv[ir]   = 9 / ||L[:, ir]||         (= exp(-0.5*ln(nrm2/81)))
    E          = exp(L * rinv)            (softmax numerator; denominator
                                           cancels in the cosine below)
    num[ir]    = sum_w E * A              (already has the 1/||img|| folded)
    u2[ir]     = sum_w E * (G @ E),  G = R_c R_c^T
    rs[ir]     = num / sqrt(u2)           (= row_sim of the reference)
    score[c,i] = sum_{r in image i} exp(6 * rs)   (log/6 applied on host)
  Final hinge-loss reduction over the 96x96 score matrix is done on host.

Sharding: captions sorted by length, dealt round-robin to 8 cores (slot s of
core k = sorted[s*8+k]) so every core shares the same per-slot padded length,
then slots are FFD-bin-packed into <=128-partition groups. One SPMD program;
per-core data (packed recipes, Gram blocks, masks) differs.

Structure (per core): fused loop over (chunk j of 432 ir-columns) x (group g).
Pass 1 per j: A matmul -> Prelu(L) -> Lsq -> nrm2 accumulated via capmask
matmul into partition rows 12j..12j+12 of one stacked [96,432] PSUM tile.
rinv per j on the [12,432] slice; broadcast to word rows via stride-0 DMA.
Pass 2 per j: T = L*rb, E = exp(T), F = G@E, P1 = E*A, P2 = E*F, then capmask
matmuls accumulate num/u2 into stacked [96,432] PSUM tiles (rows 12j..).
One epilogue on the stacked [96,432] tiles replaces 8 per-chunk epilogues.
"""

import sys

sys.path.insert(0, "/opt/trn_rl_repo")

import numpy as np

I, R, D = 96, 36, 256
C, W = 96, 48
IR = I * R  # 3456
N_CORES = 8
CPC = C // N_CORES  # captions per core = 12
CHUNK = 432  # IR columns per iteration = 12 images = one PSUM bank
N_CHUNKS = IR // CHUNK  # 8
IMG_PER_CHUNK = CHUNK // R  # 12

_CACHE = {}


def _plan_groups(cap_lens):
    """Slot lengths (max over cores per round-robin slot) + FFD packing."""
    order = np.argsort(-cap_lens, kind="stable")  # longest first
    slot_len = [int(cap_lens[order[s * N_CORES]]) for s in range(CPC)]
    bins = []  # [rows_used, [slot indices]]
    for s in range(CPC):
        ln = slot_len[s]
        for b in bins:
            if b[0] + ln <= 128:
                b[1].append(s)
                b[0] += ln
                break
        else:
            bins.append([ln, [s]])
    groups = []
    for rows, slots in bins:
        offs = []
        off = 0
        for s in slots:
            offs.append(off)
            off += slot_len[s]
        groups.append({"slots": slots, "offs": offs, "P": rows})
    return order, slot_len, groups


def _patch_act_tables():
    """Pin every activation we use to the natural_log_exp_and_others table
    set so the kernel needs exactly one ACT_TABLE_LOAD."""
    import concourse.hw_specs as hw_specs

    if getattr(hw_specs, "_act_tables_pinned", False):
        return
    orig = hw_specs.get_activation_tables

    def pinned(module_arch):
        tables = orig(module_arch)
        keep = "natural_log_exp_and_others"
        if keep in tables:
            shared = tables[keep]
            for name, funcs in tables.items():
                if name != keep:
                    tables[name] = funcs - shared
        return tables

    hw_specs.get_activation_tables = pinned
    import concourse.bacc as bacc_mod
    if getattr(bacc_mod, "get_activation_tables", None) is orig:
        bacc_mod.get_activation_tables = pinned
    hw_specs._act_tables_pinned = True


def _build_program(cap_lens):
    import concourse.bacc as bacc
    import concourse.mybir as mybir
    from concourse.tile import TileContext

    _patch_act_tables()

    fp32 = mybir.dt.float32
    f16 = mybir.dt.float16
    bf16 = mybir.dt.bfloat16
    ACT = mybir.ActivationFunctionType
    ALU = mybir.AluOpType
    AX = mybir.AxisListType

    order, slot_len, groups = _plan_groups(cap_lens)
    NG = len(groups)

    nc = bacc.Bacc("TRN2", target_bir_lowering=False, debug=False,
                   num_devices=N_CORES)

    # [chunk, 128, kc, 432] so one DMA per chunk covers both D-halves with
    # 1728B contiguous lines
    imagesT_d = nc.dram_tensor("imagesT", [N_CHUNKS, 128, 2 * CHUNK], f16,
                               kind="ExternalInput")
    cst_d, cmb_d = [], []
    for g, gr in enumerate(groups):
        P = gr["P"]
        # packed per-group f16 constants: recT [128, 2P] | G [128, P] | cmh
        # [128, CPC] (G/cmh zero-padded to 128 rows)
        cst_d.append(nc.dram_tensor(f"cst{g}", [128, 3 * P + CPC], f16,
                                    kind="ExternalInput"))
        # per-chunk stationary [P, 96] with the capmask block shifted to
        # columns 12j so the num/u2 matmuls write partition base 0 of the
        # stacked accumulators (PE requires out base partition 0/32/64)
        cmb_d.append(nc.dram_tensor(f"cmb{g}", [128, N_CHUNKS * C], bf16,
                                    kind="ExternalInput"))
    out_d = nc.dram_tensor("scores", [C, IMG_PER_CHUNK], fp32,
                           kind="ExternalOutput")

    with TileContext(nc) as tc:
        with (
            tc.tile_pool(name="const", bufs=1) as cpool,
            tc.tile_pool(name="work", bufs=4) as wpool,
            tc.tile_pool(name="rb", bufs=2) as rbpool,
            tc.tile_pool(name="small", bufs=2) as spool,
            tc.tile_pool(name="dscr", bufs=2, space="DRAM") as dpool,
            tc.tile_pool(name="psA", bufs=3, space="PSUM") as psA,
            tc.tile_pool(name="psF", bufs=2, space="PSUM") as psF,
            tc.tile_pool(name="psN", bufs=1, space="PSUM") as psN,
            tc.tile_pool(name="psAcc", bufs=1, space="PSUM") as psAcc,
        ):
            # ---- resident constants ----
            # group-0 constants first so iteration (j=0, g=0) can start early
            cst, cmb = [None] * NG, [None] * NG
            for g, gr in enumerate(groups):
                P = gr["P"]
                ct = cpool.tile([128, 3 * P + CPC], f16, tag=f"cst{g}")
                nc.sync.dma_start(out=ct[:, :], in_=cst_d[g][:, :])
                cst[g] = ct
                mb = cpool.tile([128, N_CHUNKS * C], bf16, tag=f"cmb{g}")
                nc.sync.dma_start(out=mb[:, :], in_=cmb_d[g][:, :])
                cmb[g] = mb
            recT = [cst[g][:, :2 * gr["P"]] for g, gr in enumerate(groups)]
            Gt = [cst[g][:gr["P"], 2 * gr["P"]:3 * gr["P"]]
                  for g, gr in enumerate(groups)]
            cmh = [cst[g][:gr["P"], 3 * gr["P"]:3 * gr["P"] + CPC]
                   for g, gr in enumerate(groups)]
            # images: per-chunk pieces so compute starts after the first lands
            imgT = cpool.tile([128, N_CHUNKS, 2 * CHUNK], f16, tag="imgT")
            for j in range(N_CHUNKS):
                nc.sync.dma_start(out=imgT[:, j, :], in_=imagesT_d[j, :, :])

            scores = cpool.tile([C, IMG_PER_CHUNK], fp32, tag="scores")
            accU = psAcc.tile([C, CHUNK], fp32, tag="accU")
            accM = psAcc.tile([C, CHUNK], fp32, tag="accM")

            for j in range(N_CHUNKS):
                j0 = j * CHUNK
                c0 = j * C
                As, Ls = [], []
                accN = psN.tile([CPC, CHUNK], fp32, tag="accN")
                # ---- pass 1: raw attention, leaky, norms ----
                for g, gr in enumerate(groups):
                    P = gr["P"]
                    A_ps = psA.tile([128, CHUNK], fp32, tag="A")
                    for kc in range(2):
                        nc.tensor.matmul(
                            A_ps[:P, :],
                            recT[g][:, kc * P:(kc + 1) * P],
                            imgT[:, j, kc * CHUNK:(kc + 1) * CHUNK],
                            start=(kc == 0), stop=(kc == 1))
                    As.append(A_ps)
                    L = wpool.tile([128, CHUNK], f16, tag=f"L{g}")
                    nc.scalar.activation(L[:P, :], A_ps[:P, :], ACT.Prelu,
                                         alpha=0.1)
                    Ls.append(L)
                    Lsq = wpool.tile([128, CHUNK], f16, tag="Lsq")
                    nc.gpsimd.tensor_mul(Lsq[:P, :], L[:P, :], L[:P, :])
                    nc.tensor.matmul(accN[:, :], cmh[g][:, :],
                                     Lsq[:P, :], start=(g == 0),
                                     stop=(g == NG - 1))
                # ---- rinv for this chunk: 9/sqrt(nrm2) ----
                lnt = spool.tile([CPC, CHUNK], fp32, tag="lnt")
                nc.scalar.activation(lnt[:, :], accN[:, :], ACT.Ln,
                                     scale=1.0 / 81.0)
                rinv = spool.tile([CPC, CHUNK], f16, tag="rinv")
                nc.scalar.activation(rinv[:, :], lnt[:, :], ACT.Exp,
                                     scale=-0.5)
                # broadcast rinv rows to word rows: bounce through a DRAM
                # scratch (SBUF DMA APs forbid stride-0 partition dims; DRAM
                # APs don't), then stride-0 read per slot
                rscr = dpool.tile([CPC, CHUNK], f16, tag="rscr")
                nc.sync.dma_start(out=rscr[:, :], in_=rinv[:, :])
                rbs = []
                for g, gr in enumerate(groups):
                    P = gr["P"]
                    rb = rbpool.tile([128, CHUNK], f16, tag=f"rb{g}")
                    for s, off in zip(gr["slots"], gr["offs"]):
                        lp = slot_len[s]
                        nc.sync.dma_start(
                            out=rb[off:off + lp, :],
                            in_=rscr[s:s + 1, :].to_broadcast([lp, CHUNK]))
                    rbs.append(rb)
                # ---- pass 2: softmax numerator, Gram products, reductions ----
                for g, gr in enumerate(groups):
                    P = gr["P"]
                    T = wpool.tile([128, CHUNK], f16, tag="T")
                    if g == 0:
                        nc.vector.tensor_mul(T[:P, :], Ls[g][:P, :],
                                             rbs[g][:P, :])
                    else:
                        nc.gpsimd.tensor_mul(T[:P, :], Ls[g][:P, :],
                                             rbs[g][:P, :])
                    E = wpool.tile([128, CHUNK], f16, tag="E")
                    nc.scalar.activation(E[:P, :], T[:P, :], ACT.Exp)
                    F_ps = psF.tile([128, CHUNK], fp32, tag="F")
                    nc.tensor.matmul(F_ps[:P, :], Gt[g][:, :], E[:P, :],
                                     start=True, stop=True)
                    P1 = wpool.tile([128, CHUNK], bf16, tag="P1")
                    nc.vector.tensor_mul(P1[:P, :], E[:P, :], As[g][:P, :])
                    P2 = wpool.tile([128, CHUNK], bf16, tag="P2")
                    nc.vector.tensor_mul(P2[:P, :], E[:P, :], F_ps[:P, :])
                    nc.tensor.matmul(accM[:, :], cmb[g][:P, c0:c0 + C],
                                     P1[:P, :], start=(j == 0 and g == 0),
                                     stop=(j == N_CHUNKS - 1 and g == NG - 1))
                    nc.tensor.matmul(accU[:, :], cmb[g][:P, c0:c0 + C],
                                     P2[:P, :], start=(j == 0 and g == 0),
                                     stop=(j == N_CHUNKS - 1 and g == NG - 1))
            # ---- one stacked epilogue over all (caption, chunk) rows ----
            # rs = num * rsqrt(u2); scores = sum_r exp(6*rs)
            lw = spool.tile([C, CHUNK], fp32, tag="lw")
            nc.scalar.activation(lw[:, :], accU[:, :], ACT.Ln)
            q = spool.tile([C, CHUNK], fp32, tag="q")
            nc.scalar.activation(q[:, :], lw[:, :], ACT.Exp, scale=-0.5)
            rs = spool.tile([C, CHUNK], fp32, tag="rs")
            nc.vector.tensor_mul(rs[:, :], accM[:, :], q[:, :])
            e6 = spool.tile([C, CHUNK], fp32, tag="e6")
            nc.scalar.activation(e6[:, :], rs[:, :], ACT.Exp, scale=6.0)
            nc.vector.tensor_reduce(
                scores[:, :],
                e6[:, :].rearrange("p (i r) -> p i r", r=R),
                axis=AX.X, op=ALU.add)
            nc.sync.dma_start(out=out_d[:, :], in_=scores[:, :])
    nc.compile()
    return nc, order, slot_len, groups


def _host_inputs(images, recipes, cap_lens, order, slot_len, groups):
    import ml_dtypes

    f16 = np.float16
    imgf = images.reshape(IR, D).astype(np.float64)
    n1 = np.sqrt((imgf ** 2).sum(axis=1))
    imgs = (imgf / np.maximum(n1, 1e-8)[:, None]).astype(np.float32)
    # [chunk, 128, kc*432]: imgs.T is [D, IR] = [2*128, 8*432]
    imagesT = np.ascontiguousarray(
        imgs.T.reshape(2, 128, N_CHUNKS, CHUNK).transpose(2, 1, 0, 3)
        .reshape(N_CHUNKS, 128, 2 * CHUNK)).astype(f16)

    in_maps = []
    for k in range(N_CORES):
        m = {"imagesT": imagesT}
        for g, gr in enumerate(groups):
            P = gr["P"]
            Rg = np.zeros((P, D), np.float32)  # packed, zero-padded recipes
            cm = np.zeros((P, CPC), np.float32)
            Gm = np.zeros((P, P), np.float32)
            for s, off in zip(gr["slots"], gr["offs"]):
                cap = int(order[s * N_CORES + k])
                ln = int(cap_lens[cap])
                lp = slot_len[s]
                rws = recipes[cap, :ln, :].astype(np.float32)
                Rg[off:off + ln, :] = rws
                Gm[off:off + ln, off:off + ln] = rws @ rws.T
                cm[off:off + lp, s] = 1.0
            # packed f16 constants: recT [128, 2P] | G (pad 128) | cmh (pad)
            cst = np.zeros((128, 3 * P + CPC), np.float32)
            cst[:, :2 * P] = Rg.T.reshape(2, 128, P).transpose(
                1, 0, 2).reshape(128, 2 * P)
            cst[:P, 2 * P:3 * P] = Gm
            cst[:P, 3 * P:] = cm
            m[f"cst{g}"] = cst.astype(f16)
            # stacked per-chunk stationaries: chunk j's mask block lives at
            # columns j*96 + 12j .. so its matmul writes rows 12j..12j+12 of
            # the stacked [96, CHUNK] accumulators (base partition 0)
            cms = np.zeros((128, N_CHUNKS * C), np.float32)
            for j in range(N_CHUNKS):
                cms[:P, j * C + j * CPC: j * C + (j + 1) * CPC] = cm
            m[f"cmb{g}"] = cms.astype(ml_dtypes.bfloat16)
        in_maps.append(m)
    return in_maps


def run_sharded(images, recipes, cap_lens, **spmd_kwargs):
    """Compile (cached), run on 8 cores, return (sumexp (C, I) fp64, results)."""
    from concourse.bass_utils import run_bass_kernel_spmd

    cap_lens = np.asarray(cap_lens).astype(np.int32)
    key = cap_lens.tobytes()
    if key not in _CACHE:
        _CACHE[key] = _build_program(cap_lens)
    nc, order, slot_len, groups = _CACHE[key]

    in_maps = _host_inputs(np.asarray(images), np.asarray(recipes), cap_lens,
                           order, slot_len, groups)
    res = run_bass_kernel_spmd(nc, in_maps, list(range(N_CORES)), **spmd_kwargs)

    sumexp = np.zeros((C, I), np.float64)
    for k in range(N_CORES):
        sc = res.results[k]["scores"].astype(np.float64)  # [96, 12]
        for s in range(CPC):
            cap = int(order[s * N_CORES + k])
            for j in range(N_CHUNKS):
                sumexp[cap, j * IMG_PER_CHUNK:(j + 1) * IMG_PER_CHUNK] = \
                    sc[j * CPC + s]
    return sumexp, res


def kernel(images, recipes, cap_lens):
    sumexp, _ = run_sharded(images, recipes, cap_lens)
    S = (np.log(sumexp) / 6.0).T  # (I, C)
    diag = np.diag(S)
    eye = np.eye(I, dtype=bool)
    ci = np.where(eye, 0.0, np.maximum(0.2 + S - diag[None, :], 0.0))
    cr = np.where(eye, 0.0, np.maximum(0.2 + S - diag[:, None], 0.0))
    return np.float32(ci.sum() + cr.sum())


# revision 22
# speedup vs baseline: 1.4650x; 1.2319x over previous
"""Trainium2 Bass kernel for the SCAN cross-attention contrastive loss.

Math (validated against the reference):
  For caption c with words zeroed beyond its true length, images pre-scaled
  on host by 1/||img_ir|| (column scale commutes through the whole chain):
    A[w, ir]   = <recipes[c, w, :], images_flat[ir, :]/||img_ir||>
    L          = leaky_relu(A, 0.1)       (zero rows stay zero)
    rinv[ir]   = 9 / ||L[:, ir]||         (= exp(-0.5*ln(nrm2/81)))
    E          = exp(L * rinv)            (softmax numerator; denominator
                                           cancels in the cosine below)
    num[ir]    = sum_w E * A              (already has the 1/||img|| folded)
    u2[ir]     = sum_w E * (G @ E),  G = R_c R_c^T
    rs[ir]     = num / sqrt(u2)           (= row_sim of the reference)
    score[c,i] = sum_{r in image i} exp(6 * rs)   (log/6 applied on host)
  Final hinge-loss reduction over the 96x96 score matrix is done on host.

Sharding: captions sorted by length, dealt round-robin to 8 cores (slot s of
core k = sorted[s*8+k]) so every core shares the same per-slot padded length,
then slots are FFD-bin-packed into <=128-partition groups. One SPMD program;
per-core data (packed recipes, Gram blocks, masks) differs.

Structure (per core): fused loop over (chunk j of 432 ir-columns) x (group g).
Pass 1 per j: A matmul -> Prelu(L) -> Lsq -> nrm2 accumulated via capmask
matmul into partition rows 12j..12j+12 of one stacked [96,432] PSUM tile.
rinv per j on the [12,432] slice; broadcast to word rows via stride-0 DMA.
Pass 2 per j: T = L*rb, E = exp(T), F = G@E, P1 = E*A, P2 = E*F, then capmask
matmuls accumulate num/u2 into stacked [96,432] PSUM tiles (rows 12j..).
One epilogue on the stacked [96,432] tiles replaces 8 per-chunk epilogues.
"""

import sys

sys.path.insert(0, "/opt/trn_rl_repo")

import numpy as np

I, R, D = 96, 36, 256
C, W = 96, 48
IR = I * R  # 3456
N_CORES = 8
CPC = C // N_CORES  # captions per core = 12
CHUNK = 432  # IR columns per iteration = 12 images = one PSUM bank
N_CHUNKS = IR // CHUNK  # 8
IMG_PER_CHUNK = CHUNK // R  # 12

_CACHE = {}


def _plan_groups(cap_lens):
    """Slot lengths (max over cores per round-robin slot) + FFD packing."""
    order = np.argsort(-cap_lens, kind="stable")  # longest first
    slot_len = [int(cap_lens[order[s * N_CORES]]) for s in range(CPC)]
    bins = []  # [rows_used, [slot indices]]
    for s in range(CPC):
        ln = slot_len[s]
        for b in bins:
            if b[0] + ln <= 128:
                b[1].append(s)
                b[0] += ln
                break
        else:
            bins.append([ln, [s]])
    groups = []
    for rows, slots in bins:
        offs = []
        off = 0
        for s in slots:
            offs.append(off)
            off += slot_len[s]
        groups.append({"slots": slots, "offs": offs, "P": rows})
    return order, slot_len, groups


def _patch_act_tables():
    """Pin every activation we use to the natural_log_exp_and_others table
    set so the kernel needs exactly one ACT_TABLE_LOAD."""
    import concourse.hw_specs as hw_specs

    if getattr(hw_specs, "_act_tables_pinned", False):
        return
    orig = hw_specs.get_activation_tables

    def pinned(module_arch):
        tables = orig(module_arch)
        keep = "natural_log_exp_and_others"
        if keep in tables:
            shared = tables[keep]
            for name, funcs in tables.items():
                if name != keep:
                    tables[name] = funcs - shared
        return tables

    hw_specs.get_activation_tables = pinned
    import concourse.bacc as bacc_mod
    if getattr(bacc_mod, "get_activation_tables", None) is orig:
        bacc_mod.get_activation_tables = pinned
    hw_specs._act_tables_pinned = True


def _build_program(cap_lens):
    import concourse.bacc as bacc
    import concourse.mybir as mybir
    from concourse.tile import TileContext

    _patch_act_tables()

    fp32 = mybir.dt.float32
    f16 = mybir.dt.float16
    bf16 = mybir.dt.bfloat16
    ACT = mybir.ActivationFunctionType
    ALU = mybir.AluOpType
    AX = mybir.AxisListType

    order, slot_len, groups = _plan_groups(cap_lens)
    NG = len(groups)

    nc = bacc.Bacc("TRN2", target_bir_lowering=False, debug=False,
                   num_devices=N_CORES)

    # [chunk, 128, kc, 432] so one DMA per chunk covers both D-halves with
    # 1728B contiguous lines
    imagesT_d = nc.dram_tensor("imagesT", [N_CHUNKS, 128, 2 * CHUNK], f16,
                               kind="ExternalInput")
    cst_d, cmb_d = [], []
    for g, gr in enumerate(groups):
        P = gr["P"]
        # packed per-group f16 constants: recT [128, 2P] | G [128, P] | cmh
        # [128, CPC] | bct [128, P] (padded to 128 rows)
        cst_d.append(nc.dram_tensor(f"cst{g}", [128, 4 * P + CPC], f16,
                                    kind="ExternalInput"))
        # per-chunk stationary [P, 96] with the capmask block shifted to
        # columns 12j so the num/u2 matmuls write partition base 0 of the
        # stacked accumulators (PE requires out base partition 0/32/64)
        cmb_d.append(nc.dram_tensor(f"cmb{g}", [128, N_CHUNKS * C], bf16,
                                    kind="ExternalInput"))
    out_d = nc.dram_tensor("scores", [C, IMG_PER_CHUNK], fp32,
                           kind="ExternalOutput")

    with TileContext(nc) as tc:
        with (
            tc.tile_pool(name="const", bufs=1) as cpool,
            tc.tile_pool(name="work", bufs=4) as wpool,
            tc.tile_pool(name="rb", bufs=2) as rbpool,
            tc.tile_pool(name="small", bufs=2) as spool,
            tc.tile_pool(name="dscr", bufs=2, space="DRAM") as dpool,
            tc.tile_pool(name="psA", bufs=3, space="PSUM") as psA,
            tc.tile_pool(name="psF", bufs=2, space="PSUM") as psF,
            tc.tile_pool(name="psN", bufs=1, space="PSUM") as psN,
            tc.tile_pool(name="psAcc", bufs=1, space="PSUM") as psAcc,
        ):
            # ---- resident constants ----
            # group-0 constants first so iteration (j=0, g=0) can start early
            cst, cmb = [None] * NG, [None] * NG
            for g, gr in enumerate(groups):
                P = gr["P"]
                ct = cpool.tile([128, 4 * P + CPC], f16, tag=f"cst{g}")
                nc.sync.dma_start(out=ct[:, :], in_=cst_d[g][:, :])
                cst[g] = ct
                mb = cpool.tile([128, N_CHUNKS * C], bf16, tag=f"cmb{g}")
                nc.sync.dma_start(out=mb[:, :], in_=cmb_d[g][:, :])
                cmb[g] = mb
            recT = [cst[g][:, :2 * gr["P"]] for g, gr in enumerate(groups)]
            Gt = [cst[g][:gr["P"], 2 * gr["P"]:3 * gr["P"]]
                  for g, gr in enumerate(groups)]
            cmh = [cst[g][:gr["P"], 3 * gr["P"]:3 * gr["P"] + CPC]
                   for g, gr in enumerate(groups)]
            bct = [cst[g][:CPC, 3 * gr["P"] + CPC:4 * gr["P"] + CPC]
                   for g, gr in enumerate(groups)]
            # images: per-chunk pieces so compute starts after the first lands
            imgT = cpool.tile([128, N_CHUNKS, 2 * CHUNK], f16, tag="imgT")
            for j in range(N_CHUNKS):
                nc.sync.dma_start(out=imgT[:, j, :], in_=imagesT_d[j, :, :])

            scores = cpool.tile([C, IMG_PER_CHUNK], fp32, tag="scores")
            accU = psAcc.tile([C, CHUNK], fp32, tag="accU")
            accM = psAcc.tile([C, CHUNK], fp32, tag="accM")

            for j in range(N_CHUNKS):
                j0 = j * CHUNK
                c0 = j * C
                As, Ls = [], []
                accN = psN.tile([CPC, CHUNK], fp32, tag="accN")
                # ---- pass 1: raw attention, leaky, norms ----
                for g, gr in enumerate(groups):
                    P = gr["P"]
                    A_ps = psA.tile([128, CHUNK], fp32, tag="A")
                    for kc in range(2):
                        nc.tensor.matmul(
                            A_ps[:P, :],
                            recT[g][:, kc * P:(kc + 1) * P],
                            imgT[:, j, kc * CHUNK:(kc + 1) * CHUNK],
                            start=(kc == 0), stop=(kc == 1))
                    As.append(A_ps)
                    L = wpool.tile([128, CHUNK], f16, tag=f"L{g}")
                    nc.scalar.activation(L[:P, :], A_ps[:P, :], ACT.Prelu,
                                         alpha=0.1)
                    Ls.append(L)
                    Lsq = wpool.tile([128, CHUNK], f16, tag="Lsq")
                    nc.gpsimd.tensor_mul(Lsq[:P, :], L[:P, :], L[:P, :])
                    nc.tensor.matmul(accN[:, :], cmh[g][:, :],
                                     Lsq[:P, :], start=(g == 0),
                                     stop=(g == NG - 1))
                # ---- rinv for this chunk: 9/sqrt(nrm2) ----
                lnt = spool.tile([CPC, CHUNK], fp32, tag="lnt")
                nc.scalar.activation(lnt[:, :], accN[:, :], ACT.Ln,
                                     scale=1.0 / 81.0)
                rinv = spool.tile([CPC, CHUNK], f16, tag="rinv")
                nc.scalar.activation(rinv[:, :], lnt[:, :], ACT.Exp,
                                     scale=-0.5)
                # ---- pass 2: softmax numerator, Gram products, reductions ----
                for g, gr in enumerate(groups):
                    P = gr["P"]
                    # broadcast rinv to word rows via slot-selector matmul
                    rb_ps = psF.tile([128, CHUNK], fp32, tag="mm2")
                    nc.tensor.matmul(rb_ps[:P, :], bct[g][:, :], rinv[:, :],
                                     start=True, stop=True)
                    T = wpool.tile([128, CHUNK], f16, tag="T")
                    nc.vector.tensor_mul(T[:P, :], Ls[g][:P, :], rb_ps[:P, :])
                    E = wpool.tile([128, CHUNK], f16, tag="E")
                    nc.scalar.activation(E[:P, :], T[:P, :], ACT.Exp)
                    F_ps = psF.tile([128, CHUNK], fp32, tag="mm2")
                    nc.tensor.matmul(F_ps[:P, :], Gt[g][:, :], E[:P, :],
                                     start=True, stop=True)
                    P1 = wpool.tile([128, CHUNK], bf16, tag="P1")
                    nc.vector.tensor_mul(P1[:P, :], E[:P, :], As[g][:P, :])
                    P2 = wpool.tile([128, CHUNK], bf16, tag="P2")
                    nc.vector.tensor_mul(P2[:P, :], E[:P, :], F_ps[:P, :])
                    nc.tensor.matmul(accM[:, :], cmb[g][:P, c0:c0 + C],
                                     P1[:P, :], start=(j == 0 and g == 0),
                                     stop=(j == N_CHUNKS - 1 and g == NG - 1))
                    nc.tensor.matmul(accU[:, :], cmb[g][:P, c0:c0 + C],
                                     P2[:P, :], start=(j == 0 and g == 0),
                                     stop=(j == N_CHUNKS - 1 and g == NG - 1))
            # ---- one stacked epilogue over all (caption, chunk) rows ----
            # rs = num * rsqrt(u2); scores = sum_r exp(6*rs)
            lw = spool.tile([C, CHUNK], fp32, tag="lw")
            nc.scalar.activation(lw[:, :], accU[:, :], ACT.Ln)
            q = spool.tile([C, CHUNK], fp32, tag="q")
            nc.scalar.activation(q[:, :], lw[:, :], ACT.Exp, scale=-0.5)
            rs = spool.tile([C, CHUNK], fp32, tag="rs")
            nc.vector.tensor_mul(rs[:, :], accM[:, :], q[:, :])
            e6 = spool.tile([C, CHUNK], fp32, tag="e6")
            nc.scalar.activation(e6[:, :], rs[:, :], ACT.Exp, scale=6.0)
            nc.vector.tensor_reduce(
                scores[:, :],
                e6[:, :].rearrange("p (i r) -> p i r", r=R),
                axis=AX.X, op=ALU.add)
            nc.sync.dma_start(out=out_d[:, :], in_=scores[:, :])
    nc.compile()
    return nc, order, slot_len, groups


def _host_inputs(images, recipes, cap_lens, order, slot_len, groups):
    import ml_dtypes

    f16 = np.float16
    imgf = images.reshape(IR, D).astype(np.float64)
    n1 = np.sqrt((imgf ** 2).sum(axis=1))
    imgs = (imgf / np.maximum(n1, 1e-8)[:, None]).astype(np.float32)
    # [chunk, 128, kc*432]: imgs.T is [D, IR] = [2*128, 8*432]
    imagesT = np.ascontiguousarray(
        imgs.T.reshape(2, 128, N_CHUNKS, CHUNK).transpose(2, 1, 0, 3)
        .reshape(N_CHUNKS, 128, 2 * CHUNK)).astype(f16)

    in_maps = []
    for k in range(N_CORES):
        m = {"imagesT": imagesT}
        for g, gr in enumerate(groups):
            P = gr["P"]
            Rg = np.zeros((P, D), np.float32)  # packed, zero-padded recipes
            cm = np.zeros((P, CPC), np.float32)
            Gm = np.zeros((P, P), np.float32)
            for s, off in zip(gr["slots"], gr["offs"]):
                cap = int(order[s * N_CORES + k])
                ln = int(cap_lens[cap])
                lp = slot_len[s]
                rws = recipes[cap, :ln, :].astype(np.float32)
                Rg[off:off + ln, :] = rws
                Gm[off:off + ln, off:off + ln] = rws @ rws.T
                cm[off:off + lp, s] = 1.0
            # packed f16 constants: recT [128, 2P] | G | cmh | bct (padded)
            cst = np.zeros((128, 4 * P + CPC), np.float32)
            cst[:, :2 * P] = Rg.T.reshape(2, 128, P).transpose(
                1, 0, 2).reshape(128, 2 * P)
            cst[:P, 2 * P:3 * P] = Gm
            cst[:P, 3 * P:3 * P + CPC] = cm
            cst[:CPC, 3 * P + CPC:] = cm.T
            m[f"cst{g}"] = cst.astype(f16)
            # stacked per-chunk stationaries: chunk j's mask block lives at
            # columns j*96 + 12j .. so its matmul writes rows 12j..12j+12 of
            # the stacked [96, CHUNK] accumulators (base partition 0)
            cms = np.zeros((128, N_CHUNKS * C), np.float32)
            for j in range(N_CHUNKS):
                cms[:P, j * C + j * CPC: j * C + (j + 1) * CPC] = cm
            m[f"cmb{g}"] = cms.astype(ml_dtypes.bfloat16)
        in_maps.append(m)
    return in_maps


def run_sharded(images, recipes, cap_lens, **spmd_kwargs):
    """Compile (cached), run on 8 cores, return (sumexp (C, I) fp64, results)."""
    from concourse.bass_utils import run_bass_kernel_spmd

    cap_lens = np.asarray(cap_lens).astype(np.int32)
    key = cap_lens.tobytes()
    if key not in _CACHE:
        _CACHE[key] = _build_program(cap_lens)
    nc, order, slot_len, groups = _CACHE[key]

    in_maps = _host_inputs(np.asarray(images), np.asarray(recipes), cap_lens,
                           order, slot_len, groups)
    res = run_bass_kernel_spmd(nc, in_maps, list(range(N_CORES)), **spmd_kwargs)

    sumexp = np.zeros((C, I), np.float64)
    for k in range(N_CORES):
        sc = res.results[k]["scores"].astype(np.float64)  # [96, 12]
        for s in range(CPC):
            cap = int(order[s * N_CORES + k])
            for j in range(N_CHUNKS):
                sumexp[cap, j * IMG_PER_CHUNK:(j + 1) * IMG_PER_CHUNK] = \
                    sc[j * CPC + s]
    return sumexp, res


def kernel(images, recipes, cap_lens):
    sumexp, _ = run_sharded(images, recipes, cap_lens)
    S = (np.log(sumexp) / 6.0).T  # (I, C)
    diag = np.diag(S)
    eye = np.eye(I, dtype=bool)
    ci = np.where(eye, 0.0, np.maximum(0.2 + S - diag[None, :], 0.0))
    cr = np.where(eye, 0.0, np.maximum(0.2 + S - diag[:, None], 0.0))
    return np.float32(ci.sum() + cr.sum())


# revision 24
# speedup vs baseline: 1.7439x; 1.1903x over previous
"""Trainium2 Bass kernel for the SCAN cross-attention contrastive loss.

Math (validated against the reference):
  For caption c with words zeroed beyond its true length, images pre-scaled
  on host by 1/||img_ir|| (column scale commutes through the whole chain):
    A[w, ir]   = <recipes[c, w, :], images_flat[ir, :]/||img_ir||>
    L          = leaky_relu(A, 0.1)       (zero rows stay zero)
    rinv[ir]   = 9 / ||L[:, ir]||         (= exp(-0.5*ln(nrm2/81)))
    E          = exp(L * rinv)            (softmax numerator; denominator
                                           cancels in the cosine below)
    num[ir]    = sum_w E * A              (already has the 1/||img|| folded)
    u2[ir]     = sum_w E * (G @ E),  G = R_c R_c^T
    rs[ir]     = num / sqrt(u2)           (= row_sim of the reference)
    score[c,i] = sum_{r in image i} exp(6 * rs)   (log/6 applied on host)
  Final hinge-loss reduction over the 96x96 score matrix is done on host.

Sharding: captions sorted by length, dealt round-robin to 8 cores (slot s of
core k = sorted[s*8+k]) so every core shares the same per-slot padded length,
then slots are FFD-bin-packed into <=128-partition groups. One SPMD program;
per-core data (packed recipes, Gram blocks, masks) differs.

Structure (per core): fused loop over (chunk j of 432 ir-columns) x (group g).
Pass 1 per j: A matmul -> Prelu(L) -> Lsq -> nrm2 accumulated via capmask
matmul into partition rows 12j..12j+12 of one stacked [96,432] PSUM tile.
rinv per j on the [12,432] slice; broadcast to word rows via stride-0 DMA.
Pass 2 per j: T = L*rb, E = exp(T), F = G@E, P1 = E*A, P2 = E*F, then capmask
matmuls accumulate num/u2 into stacked [96,432] PSUM tiles (rows 12j..).
One epilogue on the stacked [96,432] tiles replaces 8 per-chunk epilogues.
"""

import sys

sys.path.insert(0, "/opt/trn_rl_repo")

import numpy as np

I, R, D = 96, 36, 256
C, W = 96, 48
IR = I * R  # 3456
N_CORES = 8
CPC = C // N_CORES  # captions per core = 12
CHUNK = 432  # IR columns per iteration = 12 images = one PSUM bank
N_CHUNKS = IR // CHUNK  # 8
IMG_PER_CHUNK = CHUNK // R  # 12

_CACHE = {}


def _plan_groups(cap_lens):
    """Slot lengths (max over cores per round-robin slot) + FFD packing."""
    order = np.argsort(-cap_lens, kind="stable")  # longest first
    slot_len = [int(cap_lens[order[s * N_CORES]]) for s in range(CPC)]
    bins = []  # [rows_used, [slot indices]]
    for s in range(CPC):
        ln = slot_len[s]
        for b in bins:
            if b[0] + ln <= 128:
                b[1].append(s)
                b[0] += ln
                break
        else:
            bins.append([ln, [s]])
    groups = []
    for rows, slots in bins:
        offs = []
        off = 0
        for s in slots:
            offs.append(off)
            off += slot_len[s]
        groups.append({"slots": slots, "offs": offs, "P": rows})
    return order, slot_len, groups


def _patch_act_tables():
    """Pin every activation we use to the natural_log_exp_and_others table
    set so the kernel needs exactly one ACT_TABLE_LOAD."""
    import concourse.hw_specs as hw_specs

    if getattr(hw_specs, "_act_tables_pinned", False):
        return
    orig = hw_specs.get_activation_tables

    def pinned(module_arch):
        tables = orig(module_arch)
        keep = "natural_log_exp_and_others"
        if keep in tables:
            shared = tables[keep]
            for name, funcs in tables.items():
                if name != keep:
                    tables[name] = funcs - shared
        return tables

    hw_specs.get_activation_tables = pinned
    import concourse.bacc as bacc_mod
    if getattr(bacc_mod, "get_activation_tables", None) is orig:
        bacc_mod.get_activation_tables = pinned
    hw_specs._act_tables_pinned = True


def _build_program(cap_lens):
    import concourse.bacc as bacc
    import concourse.mybir as mybir
    from concourse.tile import TileContext

    _patch_act_tables()

    fp32 = mybir.dt.float32
    f16 = mybir.dt.float16
    bf16 = mybir.dt.bfloat16
    ACT = mybir.ActivationFunctionType
    ALU = mybir.AluOpType
    AX = mybir.AxisListType

    order, slot_len, groups = _plan_groups(cap_lens)
    NG = len(groups)

    nc = bacc.Bacc("TRN2", target_bir_lowering=False, debug=False,
                   num_devices=N_CORES)

    # [chunk, 128, kc, 432] so one DMA per chunk covers both D-halves with
    # 1728B contiguous lines
    imagesT_d = nc.dram_tensor("imagesT", [N_CHUNKS, 128, 2 * CHUNK], f16,
                               kind="ExternalInput")
    cst_d, cmb_d = [], []
    for g, gr in enumerate(groups):
        P = gr["P"]
        # packed per-group f16 constants: recT [128, 2P] | G [128, P] | cmh
        # [128, CPC] | bct [128, P] (padded to 128 rows)
        cst_d.append(nc.dram_tensor(f"cst{g}", [128, 4 * P + CPC], f16,
                                    kind="ExternalInput"))
        # per-chunk stationary [P, 96] with the capmask block shifted to
        # columns 12j so the num/u2 matmuls write partition base 0 of the
        # stacked accumulators (PE requires out base partition 0/32/64)
        cmb_d.append(nc.dram_tensor(f"cmb{g}", [128, N_CHUNKS * C], bf16,
                                    kind="ExternalInput"))
    out_d = nc.dram_tensor("scores", [C, IMG_PER_CHUNK], fp32,
                           kind="ExternalOutput")

    with TileContext(nc) as tc:
        with (
            tc.tile_pool(name="const", bufs=1) as cpool,
            tc.tile_pool(name="work", bufs=4) as wpool,
            tc.tile_pool(name="small", bufs=2) as spool,
            tc.tile_pool(name="psA", bufs=3, space="PSUM") as psA,
            tc.tile_pool(name="psF", bufs=3, space="PSUM") as psF,
            tc.tile_pool(name="psAcc", bufs=1, space="PSUM") as psAcc,
        ):
            # ---- resident constants ----
            # group-0 constants first so iteration (j=0, g=0) can start early
            cst, cmb = [None] * NG, [None] * NG
            for g, gr in enumerate(groups):
                P = gr["P"]
                ct = cpool.tile([128, 4 * P + CPC], f16, tag=f"cst{g}")
                nc.sync.dma_start(out=ct[:, :], in_=cst_d[g][:, :])
                cst[g] = ct
                mb = cpool.tile([128, N_CHUNKS * C], bf16, tag=f"cmb{g}")
                nc.sync.dma_start(out=mb[:, :], in_=cmb_d[g][:, :])
                cmb[g] = mb
            recT = [cst[g][:, :2 * gr["P"]] for g, gr in enumerate(groups)]
            Gt = [cst[g][:gr["P"], 2 * gr["P"]:3 * gr["P"]]
                  for g, gr in enumerate(groups)]
            cmh = [cst[g][:gr["P"], 3 * gr["P"]:3 * gr["P"] + CPC]
                   for g, gr in enumerate(groups)]
            bct = [cst[g][:CPC, 3 * gr["P"] + CPC:4 * gr["P"] + CPC]
                   for g, gr in enumerate(groups)]
            # images: per-chunk pieces so compute starts after the first lands
            imgT = cpool.tile([128, N_CHUNKS, 2 * CHUNK], f16, tag="imgT")
            for j in range(N_CHUNKS):
                nc.sync.dma_start(out=imgT[:, j, :], in_=imagesT_d[j, :, :])

            scores = cpool.tile([C, IMG_PER_CHUNK], fp32, tag="scores")
            accU = psAcc.tile([C, CHUNK], fp32, tag="accU")
            accM = psAcc.tile([C, CHUNK], fp32, tag="accM")

            for j in range(N_CHUNKS):
                c0 = j * C
                # ---- pass 1: raw attention, leaky, norms ----
                As, Ls = [], []
                for g, gr in enumerate(groups):
                    P = gr["P"]
                    A_ps = psA.tile([128, CHUNK], fp32, tag="A")
                    for kc in range(2):
                        nc.tensor.matmul(
                            A_ps[:P, :],
                            recT[g][:, kc * P:(kc + 1) * P],
                            imgT[:, j, kc * CHUNK:(kc + 1) * CHUNK],
                            start=(kc == 0), stop=(kc == 1))
                    As.append(A_ps)
                    L = wpool.tile([128, CHUNK], f16, tag=f"L{g}")
                    nc.scalar.activation(L[:, :], A_ps[:, :], ACT.Prelu,
                                         alpha=0.1)
                    Ls.append(L)
                Lsqs = []
                for g, gr in enumerate(groups):
                    Lsq = wpool.tile([128, CHUNK], f16, tag=f"Lsq{g}")
                    nc.gpsimd.tensor_mul(Lsq[:, :], Ls[g][:, :], Ls[g][:, :])
                    Lsqs.append(Lsq)
                accN = psF.tile([CPC, CHUNK], fp32, tag="mm2")
                for g, gr in enumerate(groups):
                    nc.tensor.matmul(accN[:, :], cmh[g][:, :],
                                     Lsqs[g][:gr["P"], :], start=(g == 0),
                                     stop=(g == NG - 1))
                # ---- rinv for this chunk: 9/sqrt(nrm2) ----
                lnt = spool.tile([CPC, CHUNK], fp32, tag="lnt")
                nc.scalar.activation(lnt[:, :], accN[:, :], ACT.Ln,
                                     scale=1.0 / 81.0)
                rinv = spool.tile([CPC, CHUNK], f16, tag="rinv")
                nc.scalar.activation(rinv[:, :], lnt[:, :], ACT.Exp,
                                     scale=-0.5)
                # ---- pass 2, stage-batched so engines pipeline across g ----
                rbs = []
                for g, gr in enumerate(groups):
                    P = gr["P"]
                    rb_ps = psF.tile([128, CHUNK], fp32, tag="mm2")
                    nc.tensor.matmul(rb_ps[:P, :], bct[g][:, :], rinv[:, :],
                                     start=True, stop=True)
                    rbs.append(rb_ps)
                # T slices share one tile so E is a single wide activation
                T3 = wpool.tile([128, NG * CHUNK], f16, tag="T3")
                for g, gr in enumerate(groups):
                    nc.vector.tensor_mul(T3[:, g * CHUNK:(g + 1) * CHUNK],
                                         Ls[g][:, :], rbs[g][:, :])
                E3 = wpool.tile([128, NG * CHUNK], f16, tag="E3")
                nc.scalar.activation(E3[:, :], T3[:, :], ACT.Exp)
                Fs = []
                for g, gr in enumerate(groups):
                    P = gr["P"]
                    F_ps = psF.tile([128, CHUNK], fp32, tag="mm2")
                    nc.tensor.matmul(F_ps[:P, :],
                                     Gt[g][:, :],
                                     E3[:P, g * CHUNK:(g + 1) * CHUNK],
                                     start=True, stop=True)
                    Fs.append(F_ps)
                P1s, P2s = [], []
                for g, gr in enumerate(groups):
                    Eg = E3[:, g * CHUNK:(g + 1) * CHUNK]
                    P1 = wpool.tile([128, CHUNK], bf16, tag=f"P1{g}")
                    nc.vector.tensor_mul(P1[:, :], Eg, As[g][:, :])
                    P1s.append(P1)
                    P2 = wpool.tile([128, CHUNK], bf16, tag=f"P2{g}")
                    nc.vector.tensor_mul(P2[:, :], Eg, Fs[g][:, :])
                    P2s.append(P2)
                for g, gr in enumerate(groups):
                    P = gr["P"]
                    nc.tensor.matmul(accM[:, :], cmb[g][:P, c0:c0 + C],
                                     P1s[g][:P, :], start=(j == 0 and g == 0),
                                     stop=(j == N_CHUNKS - 1 and g == NG - 1))
                    nc.tensor.matmul(accU[:, :], cmb[g][:P, c0:c0 + C],
                                     P2s[g][:P, :], start=(j == 0 and g == 0),
                                     stop=(j == N_CHUNKS - 1 and g == NG - 1))
            # ---- one stacked epilogue over all (caption, chunk) rows ----
            # rs = num * rsqrt(u2); scores = sum_r exp(6*rs)
            lw = spool.tile([C, CHUNK], fp32, tag="lw")
            nc.scalar.activation(lw[:, :], accU[:, :], ACT.Ln)
            q = spool.tile([C, CHUNK], fp32, tag="q")
            nc.scalar.activation(q[:, :], lw[:, :], ACT.Exp, scale=-0.5)
            rs = spool.tile([C, CHUNK], fp32, tag="rs")
            nc.vector.tensor_mul(rs[:, :], accM[:, :], q[:, :])
            e6 = spool.tile([C, CHUNK], fp32, tag="e6")
            nc.scalar.activation(e6[:, :], rs[:, :], ACT.Exp, scale=6.0)
            nc.vector.tensor_reduce(
                scores[:, :],
                e6[:, :].rearrange("p (i r) -> p i r", r=R),
                axis=AX.X, op=ALU.add)
            nc.sync.dma_start(out=out_d[:, :], in_=scores[:, :])
    nc.compile()
    return nc, order, slot_len, groups


def _host_inputs(images, recipes, cap_lens, order, slot_len, groups):
    import ml_dtypes

    f16 = np.float16
    imgf = images.reshape(IR, D).astype(np.float64)
    n1 = np.sqrt((imgf ** 2).sum(axis=1))
    imgs = (imgf / np.maximum(n1, 1e-8)[:, None]).astype(np.float32)
    # [chunk, 128, kc*432]: imgs.T is [D, IR] = [2*128, 8*432]
    imagesT = np.ascontiguousarray(
        imgs.T.reshape(2, 128, N_CHUNKS, CHUNK).transpose(2, 1, 0, 3)
        .reshape(N_CHUNKS, 128, 2 * CHUNK)).astype(f16)

    in_maps = []
    for k in range(N_CORES):
        m = {"imagesT": imagesT}
        for g, gr in enumerate(groups):
            P = gr["P"]
            Rg = np.zeros((P, D), np.float32)  # packed, zero-padded recipes
            cm = np.zeros((P, CPC), np.float32)
            Gm = np.zeros((P, P), np.float32)
            for s, off in zip(gr["slots"], gr["offs"]):
                cap = int(order[s * N_CORES + k])
                ln = int(cap_lens[cap])
                lp = slot_len[s]
                rws = recipes[cap, :ln, :].astype(np.float32)
                Rg[off:off + ln, :] = rws
                Gm[off:off + ln, off:off + ln] = rws @ rws.T
                cm[off:off + lp, s] = 1.0
            # packed f16 constants: recT [128, 2P] | G | cmh | bct (padded)
            cst = np.zeros((128, 4 * P + CPC), np.float32)
            cst[:, :2 * P] = Rg.T.reshape(2, 128, P).transpose(
                1, 0, 2).reshape(128, 2 * P)
            cst[:P, 2 * P:3 * P] = Gm
            cst[:P, 3 * P:3 * P + CPC] = cm
            cst[:CPC, 3 * P + CPC:] = cm.T
            m[f"cst{g}"] = cst.astype(f16)
            # stacked per-chunk stationaries: chunk j's mask block lives at
            # columns j*96 + 12j .. so its matmul writes rows 12j..12j+12 of
            # the stacked [96, CHUNK] accumulators (base partition 0)
            cms = np.zeros((128, N_CHUNKS * C), np.float32)
            for j in range(N_CHUNKS):
                cms[:P, j * C + j * CPC: j * C + (j + 1) * CPC] = cm
            m[f"cmb{g}"] = cms.astype(ml_dtypes.bfloat16)
        in_maps.append(m)
    return in_maps


def run_sharded(images, recipes, cap_lens, **spmd_kwargs):
    """Compile (cached), run on 8 cores, return (sumexp (C, I) fp64, results)."""
    from concourse.bass_utils import run_bass_kernel_spmd

    cap_lens = np.asarray(cap_lens).astype(np.int32)
    key = cap_lens.tobytes()
    if key not in _CACHE:
        _CACHE[key] = _build_program(cap_lens)
    nc, order, slot_len, groups = _CACHE[key]

    in_maps = _host_inputs(np.asarray(images), np.asarray(recipes), cap_lens,
                           order, slot_len, groups)
    res = run_bass_kernel_spmd(nc, in_maps, list(range(N_CORES)), **spmd_kwargs)

    sumexp = np.zeros((C, I), np.float64)
    for k in range(N_CORES):
        sc = res.results[k]["scores"].astype(np.float64)  # [96, 12]
        for s in range(CPC):
            cap = int(order[s * N_CORES + k])
            for j in range(N_CHUNKS):
                sumexp[cap, j * IMG_PER_CHUNK:(j + 1) * IMG_PER_CHUNK] = \
                    sc[j * CPC + s]
    return sumexp, res


def kernel(images, recipes, cap_lens):
    sumexp, _ = run_sharded(images, recipes, cap_lens)
    S = (np.log(sumexp) / 6.0).T  # (I, C)
    diag = np.diag(S)
    eye = np.eye(I, dtype=bool)
    ci = np.where(eye, 0.0, np.maximum(0.2 + S - diag[None, :], 0.0))
    cr = np.where(eye, 0.0, np.maximum(0.2 + S - diag[:, None], 0.0))
    return np.float32(ci.sum() + cr.sum())
